# revision 27
# baseline (speedup 1.0000x reference)
"""Trainium2 Bass kernel for a 2-layer GraphSAGE(mean) encoder (8 NeuronCores).

v2 design (replaces the dma_gather-centric v1, which was bottlenecked by
SWDGE descriptor generation at ~8ns/row on the Q7s and by the GpSimd/DVE
shared-SBUF-port contention it induced):

  - Layer 0 (dst-partitioned by dst0 % 8): the host materializes each core's
    incoming-edge message rows (the "halo of remote src features" in edge-slot
    order) as a bf16 array plus the per-chunk one-hot segment-sum matrices M
    (mean weights folded in).  The device streams both sequentially (HWDGE),
    applies log1p on the Scalar engine, and accumulates aggT = msgs^T @ M on
    the PE per 32-dst sub-tile.  No gathers, no per-chunk DVE one-hot builds.
  - Layer 1 (edges by src1 % 8, dst1 in permuted layout): h1 is written to
    DRAM in bf16; a SWDGE dma_gather stages per-edge h1 rows; partial segment
    sums are computed in transposed [f, d] layout and ReduceScatter(add)
    delivers each core its own 1250 dst1 rows.  Final projection / relu /
    normalize / heads run per owning core; the host interleaves outputs.
  - All matmul operands are bf16 (fast weight load, 1 col/cycle); PSUM
    accumulation is f32.
"""

import math

import numpy as np

import concourse.bass as bass
import concourse.bacc as bacc
import concourse.mybir as mybir
from concourse.bass_utils import run_bass_kernel_spmd
from concourse.masks import make_identity
from concourse.tile import TileContext

# ----------------------------------------------------------------------------
# Problem constants (hardcoded; the harness always uses these shapes).
# ----------------------------------------------------------------------------
N0, N1, N2 = 200000, 50000, 10000
E0, E1 = 800000, 160000
F_IN, H, L = 128, 256, 32
NC = 8
P = 128

B1 = math.ceil(N2 // NC / P) * P  # 1280 padded per-core dst1 rows
T1 = B1 // P  # 10 final tiles per core
T1P = NC * T1  # 80 permuted partial tiles
RS_GROUPS = [(0, 3), (3, 2), (5, 3), (8, 2)]  # (tt0, n): pipelined sub-RS groups

# local h1 rows: positions [0, B1) hold the core's dst1 nodes (load-balanced
# permutation), the remaining dst0 nodes follow
T0 = math.ceil((B1 + N1 // NC - N2 // NC) / P) + 1  # 51 layer-0 supertiles
R0 = T0 * P  # 6528 padded local dst rows per core (slack eases balancing)
W0 = 32  # layer-0 M sub-tile width (dst cols per chunk)
S0 = P // W0  # 4 sub-tiles per supertile
NB0 = T0 * S0  # 200 layer-0 buckets

LO_T0 = 31  # h1 rows [0, LO_T0*128) are duplicated into h1_lo for early gathers
LO_ROWS = LO_T0 * P

G0 = 64  # layer-0 chunks per staging group
G1 = 16  # layer-1 chunks per staging group
GCH = 8  # chunks per dma_gather instruction (1024 idxs)

EPS_NORM = 1e-12

f32 = mybir.dt.float32
bf16 = mybir.dt.bfloat16
i16 = mybir.dt.int16
npbf = mybir.dt.np(bf16)


def _ranks_from_sorted(keys_sorted):
    """rank of each element within its equal-key run (keys_sorted ascending)."""
    n = keys_sorted.shape[0]
    if n == 0:
        return np.zeros(0, np.int64)
    new_run = np.empty(n, bool)
    new_run[0] = True
    new_run[1:] = keys_sorted[1:] != keys_sorted[:-1]
    starts = np.flatnonzero(new_run)
    run_ids = np.cumsum(new_run) - 1
    return np.arange(n) - starts[run_ids]


def _chunk_layout(counts, n_buckets):
    """counts: [NC, n_buckets] edge counts.  Returns (cap, base, total):
    cap[b] = chunks for bucket b (max over cores, >= 1), base = cumsum."""
    cap = np.maximum(np.ceil(counts / P).astype(np.int64).max(axis=0), 1)
    base = np.zeros(n_buckets + 1, np.int64)
    np.cumsum(cap, out=base[1:])
    return cap, base, int(base[-1])


def _wrap_idx(seg):
    """int16 flat idx list -> [128, len/16] wrapped layout for dma_gather."""
    ncols = len(seg) // 16
    return np.tile(seg.reshape(ncols, 16).T, (8, 1))


class _Plan:
    """Shared (cross-core) program structure + per-core input arrays."""

    def __init__(self, x, src0, dst0, src1, dst1):
        src0 = np.asarray(src0).astype(np.int64)
        dst0 = np.asarray(dst0).astype(np.int64)
        src1 = np.asarray(src1).astype(np.int64)
        dst1 = np.asarray(dst1).astype(np.int64)
        x = np.asarray(x, np.float32)

        deg0 = np.bincount(dst0, minlength=N1)
        inv0 = (1.0 / np.maximum(deg0, 1)).astype(np.float32)
        deg1 = np.bincount(dst1, minlength=N2)
        inv1 = (1.0 / np.maximum(deg1, 1)).astype(np.float32)

        # ------- load-balanced dst1 -> (tile, slot) position per owner ------
        core1 = src1 % NC
        nodevec = np.zeros((N2, NC), np.int64)
        np.add.at(nodevec, (dst1, core1), 1)
        pos_of = np.empty(N2, np.int64)
        for o in range(NC):
            nodes = np.arange(o, N2, NC)
            vec = nodevec[nodes]
            order_n = np.argsort(-vec.sum(1), kind="stable")
            fill = np.zeros(T1, np.int64)
            load = np.zeros((T1, NC), np.int64)
            tt_of = np.empty(len(nodes), np.int64)
            slot_of = np.empty(len(nodes), np.int64)
            for i in order_n:
                v = vec[i]
                score = (load + v).max(axis=1).astype(np.float64)
                score[fill >= P] = 1e18
                t = int(np.argmin(score))
                tt_of[i] = t
                slot_of[i] = fill[t]
                load[t] += v
                fill[t] += 1
            pos_of[nodes] = tt_of * P + slot_of
        self.pos_of = pos_of

        # reorder slots within each L1 tile so the tile's four layer-0
        # buckets carry balanced in-degree (slot order is free for L1)
        deg0n = deg0  # in-degree per dst0 node id
        for o in range(NC):
            nodes = np.arange(o, N2, NC)
            tts = pos_of[nodes] // P
            for tt in range(T1):
                sel = nodes[tts == tt]
                order_n = sel[np.argsort(-deg0n[sel], kind="stable")]
                loads = np.zeros(S0)
                fill = np.zeros(S0, np.int64)
                for g in order_n:
                    cand = loads + deg0n[g]
                    cand[fill >= W0] = np.inf
                    b = int(np.argmin(cand))
                    pos_of[g] = tt * P + b * W0 + fill[b]
                    loads[b] += deg0n[g]
                    fill[b] += 1

        # layer-0 local row of each dst0 node: dst1 nodes sit at their
        # position; the remaining dst0 nodes are LPT-balanced over the
        # 32-row buckets from row B1 so bucket in-degree stays under 4*128
        ldmap = np.empty(N1, np.int64)
        nbuck = (R0 - B1) // W0
        for c in range(NC):
            ds = np.arange(c, N1, NC)
            is1 = ds < N2
            ldmap[ds[is1]] = pos_of[ds[is1]]
            rest = ds[~is1]
            order_n = rest[np.argsort(-deg0n[rest], kind="stable")]
            loads = np.zeros(nbuck)
            fill = np.zeros(nbuck, np.int64)
            for g in order_n:
                cand = loads + deg0n[g]
                cand[fill >= W0] = np.inf
                b = int(np.argmin(cand))
                ldmap[g] = B1 + b * W0 + fill[b]
                loads[b] += deg0n[g]
                fill[b] += 1
        self.ldmap = ldmap

        # ---------------- layer 0 ----------------
        core0 = dst0 % NC
        ld0 = ldmap[dst0]
        b0 = ld0 // W0  # bucket in [0, NB0)
        counts0 = np.zeros((NC, NB0), np.int64)
        np.add.at(counts0, (core0, b0), 1)
        self.cap0, self.base0, self.C0 = _chunk_layout(counts0, NB0)

        order = np.lexsort((b0, core0))
        key = core0[order] * NB0 + b0[order]
        ranks = _ranks_from_sorted(key)
        kk = self.base0[b0[order]] + ranks // P
        pp = ranks % P

        self.msgs0 = np.zeros((NC, P, self.C0, F_IN), npbf)
        self.m0 = np.zeros((NC, P, self.C0, W0), npbf)
        co = core0[order]
        so = src0[order]
        do = dst0[order]
        ldo = ld0[order]
        gathered = x[so].astype(npbf)
        self.msgs0[co, pp, kk, :] = gathered
        self.m0[co, pp, kk, (ldo % W0)] = inv0[do]

        # per-core self rows, transposed: xselfT[c][f, ldmap[d]] = x[d, f]
        self.xselfT = np.zeros((NC, F_IN, R0), npbf)
        for c in range(NC):
            ds = np.arange(c, N1, NC)
            self.xselfT[c][:, ldmap[ds]] = x[ds].T.astype(npbf)

        # ---------------- layer 1 ----------------
        r1 = ldmap[src1]  # local h1 row on owning core
        o1 = dst1 % NC
        t1 = o1 * T1 + pos_of[dst1] // P  # permuted tile in [0, T1P)
        dloc1 = pos_of[dst1] % P
        counts1 = np.zeros((NC, T1P), np.int64)
        np.add.at(counts1, (core1, t1), 1)
        cap1, _, _ = _chunk_layout(counts1, T1P)
        cmax = int(cap1.max())

        # within each (core, tile), edges sorted by src row -> per-tile chunk
        # j holds the j-th lowest src rows; chunk max-rows ascend with j
        order = np.lexsort((r1, t1, core1))
        key = core1[order] * T1P + t1[order]
        ranks = _ranks_from_sorted(key)
        jj = ranks // P
        pp = ranks % P
        co = core1[order]
        to = t1[order]
        ro = r1[order]

        maxi = np.zeros((NC, T1P, cmax), np.int64)
        np.maximum.at(maxi, (co, to, jj), ro)
        maxi_sh = maxi.max(axis=0)  # [T1P, cmax] shared across cores
        e_cnt = np.zeros(T1P, np.int64)
        for T in range(T1P):
            n = int(cap1[T])
            # early = longest prefix of chunks whose rows all fit in h1_lo
            e_cnt[T] = int(
                (np.maximum.accumulate(maxi_sh[T, :n]) < LO_ROWS).sum()
            )

        # global chunk order: all early chunks (tile-major), then late chunks
        # ordered by RS group so each sub-ReduceScatter can fire early
        def rs_group(tt):
            for k, (tt0, n) in enumerate(RS_GROUPS):
                if tt0 <= tt < tt0 + n:
                    return k
            raise AssertionError(tt)

        self.rs_group = rs_group
        lateT = sorted(range(T1P), key=lambda T: (rs_group(T % T1), T))
        orderE = [(T, j) for T in range(T1P) for j in range(e_cnt[T])]
        orderL = [(T, j) for T in lateT for j in range(e_cnt[T], int(cap1[T]))]
        chunk_id = np.full((T1P, cmax), -1, np.int64)
        for g, (T, j) in enumerate(orderE + orderL):
            chunk_id[T, j] = g
        self.CE = len(orderE)
        self.C1 = len(orderE) + len(orderL)
        self.cap1 = cap1
        self.e_cnt = e_cnt
        self.lateT = lateT
        self.echunks = [
            [int(chunk_id[T, j]) for j in range(e_cnt[T])] for T in range(T1P)
        ]
        self.lchunks = [
            [int(chunk_id[T, j]) for j in range(e_cnt[T], int(cap1[T]))]
            for T in range(T1P)
        ]

        kk = chunk_id[to, jj]
        self.m1 = np.zeros((NC, P, self.C1, P), npbf)
        self.m1[co, pp, kk, dloc1[order]] = inv1[dst1[order]]

        idx_flat = np.zeros((NC, self.C1 * P), np.int16)
        idx_flat[co, kk * P + pp] = ro.astype(np.int16)

        # gather instructions: spans of <= GCH chunks, phase-pure, never
        # crossing a G1 staging-group boundary
        self.spans = []  # (k0, n, from_lo)
        for lo, hi, from_lo in ((0, self.CE, True), (self.CE, self.C1, False)):
            k0 = lo
            while k0 < hi:
                gend = (k0 // G1 + 1) * G1
                n = min(GCH, hi - k0, gend - k0)
                self.spans.append((k0, n, from_lo))
                k0 += n
        self.idx_cols = self.C1 * P // 16
        self.idx1 = np.zeros((NC, 128, self.idx_cols), np.int16)
        for c in range(NC):
            col = 0
            for k0, n, _ in self.spans:
                seg = idx_flat[c, k0 * P : (k0 + n) * P]
                self.idx1[c, :, col : col + n * P // 16] = _wrap_idx(seg)
                col += n * P // 16

        # ---------------- weights ----------------
        self.signature = (
            tuple(self.cap0.tolist()),
            tuple(self.cap1.tolist()),
            tuple(self.e_cnt.tolist()),
        )


# ----------------------------------------------------------------------------
# Program construction
# ----------------------------------------------------------------------------
def _build_program(plan, has_b0, has_b1, has_bmu, has_bvar):
    nc = bacc.Bacc(num_devices=NC, name="gnn_sage_v2", num_swdge_queues=2)

    C0, C1 = plan.C0, plan.C1
    msgs0_d = nc.dram_tensor("msgs0", (P, C0, F_IN), bf16, kind="ExternalInput")
    m0_d = nc.dram_tensor("m0", (P, C0, W0), bf16, kind="ExternalInput")
    xselfT_d = nc.dram_tensor("xselfT", (F_IN, R0), bf16, kind="ExternalInput")
    m1_d = nc.dram_tensor("m1", (P, C1, P), bf16, kind="ExternalInput")
    idx1_d = nc.dram_tensor("idx1", (128, plan.idx_cols), i16, kind="ExternalInput")
    ws0_d = nc.dram_tensor("ws0", (F_IN, H), bf16, kind="ExternalInput")
    wn0_d = nc.dram_tensor("wn0", (F_IN, H), bf16, kind="ExternalInput")
    ws1_d = nc.dram_tensor("ws1", (2, P, H), bf16, kind="ExternalInput")
    wn1_d = nc.dram_tensor("wn1", (2, P, H), bf16, kind="ExternalInput")
    wmu_d = nc.dram_tensor("wmu", (2, P, L), bf16, kind="ExternalInput")
    wvar_d = nc.dram_tensor("wvar", (2, P, L), bf16, kind="ExternalInput")
    b_d = {}
    if has_b0:
        b_d["b0"] = nc.dram_tensor("b0", (H,), f32, kind="ExternalInput")
    if has_b1:
        b_d["b1"] = nc.dram_tensor("b1", (H,), f32, kind="ExternalInput")
    if has_bmu:
        b_d["b_mu"] = nc.dram_tensor("b_mu", (L,), f32, kind="ExternalInput")
    if has_bvar:
        b_d["b_var"] = nc.dram_tensor("b_var", (L,), f32, kind="ExternalInput")

    h1_d = nc.dram_tensor("h1_scratch", (R0, H), bf16, kind="Internal")
    h1lo_d = nc.dram_tensor("h1_lo", (LO_ROWS, H), bf16, kind="Internal")
    partials_g_d = [
        nc.dram_tensor(f"s1_partials_{k}", (NC, P, n, 2, P), bf16, kind="Internal")
        for k, (_, n) in enumerate(RS_GROUPS)
    ]
    rs_g_d = [
        nc.dram_tensor(f"s1_reduced_{k}", (P, n, 2, P), bf16, kind="Internal")
        for k, (_, n) in enumerate(RS_GROUPS)
    ]

    zloc_d = nc.dram_tensor("z_loc", (B1, L), f32, kind="ExternalOutput")
    zscale_d = nc.dram_tensor("z_scale", (B1, L), f32, kind="ExternalOutput")

    AT = mybir.ActivationFunctionType
    OP = mybir.AluOpType

    # layer-0 chunk -> (supertile, subtile, index-in-bucket, bucket-size)
    chunk0_meta = []
    for b in range(NB0):
        nb = int(plan.cap0[b])
        for i in range(nb):
            chunk0_meta.append((b // S0, b % S0, i, nb))
    with TileContext(nc, num_cores=NC) as tc:
        with (
            tc.tile_pool(name="const", bufs=1) as cp,
            tc.tile_pool(name="stage0", bufs=2) as stagep,
            tc.tile_pool(name="mstage", bufs=2) as mp,
            tc.tile_pool(name="stage1", bufs=3) as stage1p,
            tc.tile_pool(name="meta", bufs=3) as metap,
            tc.tile_pool(name="small", bufs=4) as sp,
            tc.tile_pool(name="ps_agg", bufs=2, space="PSUM") as ps_agg,
            tc.tile_pool(name="ps_tr", bufs=2, space="PSUM") as ps_tr,
            tc.tile_pool(name="ps_out", bufs=2, space="PSUM") as ps_out,
        ):
            # ---- constants ----
            ident_sb = cp.tile([P, P], bf16)
            make_identity(nc, ident_sb[:])
            ws0_sb = cp.tile([P, H], bf16)
            nc.sync.dma_start(out=ws0_sb[:], in_=ws0_d[:])
            wn0_sb = cp.tile([P, H], bf16)
            nc.sync.dma_start(out=wn0_sb[:], in_=wn0_d[:])
            ws1_sb = [cp.tile([P, H], bf16, tag=f"ws1_{k}", name=f"ws1_{k}") for k in range(2)]
            wn1_sb = [cp.tile([P, H], bf16, tag=f"wn1_{k}", name=f"wn1_{k}") for k in range(2)]
            wmu_sb = [cp.tile([P, L], bf16, tag=f"wmu_{k}", name=f"wmu_{k}") for k in range(2)]
            wvar_sb = [cp.tile([P, L], bf16, tag=f"wvar_{k}", name=f"wvar_{k}") for k in range(2)]
            for k in range(2):
                nc.sync.dma_start(out=ws1_sb[k][:], in_=ws1_d[k])
                nc.sync.dma_start(out=wn1_sb[k][:], in_=wn1_d[k])
                nc.sync.dma_start(out=wmu_sb[k][:], in_=wmu_d[k])
                nc.sync.dma_start(out=wvar_sb[k][:], in_=wvar_d[k])
            if b_d:
                ones_sb = cp.tile([1, P], f32)
                nc.vector.memset(ones_sb[:], 1.0)
                brow = {}
                for name, hd in b_d.items():
                    t = cp.tile([1, hd.shape[0]], f32, tag=f"brow_{name}", name=f"brow_{name}")
                    nc.sync.dma_start(out=t[:], in_=hd[:].rearrange("n -> 1 n"))
                    brow[name] = t

            # xselfT: load + log1p once
            xselfT_sb = cp.tile([F_IN, R0], bf16)
            nc.sync.dma_start(out=xselfT_sb[:], in_=xselfT_d[:])
            nc.scalar.activation(xselfT_sb[:], xselfT_sb[:], AT.Ln, bias=1.0)

            # h1T stash for the final layer's self path
            h1T_sb = cp.tile([P, 2, B1], bf16)

            # ================= Layer 0 =================
            ps_a = None
            for g0 in range(0, C0, G0):
                gsz = min(G0, C0 - g0)
                stage = stagep.tile([P, gsz * F_IN], bf16, tag="stage0")
                stage3 = stage[:].rearrange("p (k f) -> p k f", f=F_IN)
                nc.sync.dma_start(out=stage3, in_=msgs0_d[:, g0 : g0 + gsz, :])
                m0t = mp.tile([P, gsz * W0], bf16, tag="m0")
                m0t3 = m0t[:].rearrange("p (k w) -> p k w", w=W0)
                nc.sync.dma_start(out=m0t3, in_=m0_d[:, g0 : g0 + gsz, :])
                nc.scalar.activation(stage[:], stage[:], AT.Ln, bias=1.0)

                for kk in range(gsz):
                    t, s, i, nb = chunk0_meta[g0 + kk]
                    if s == 0 and i == 0:
                        ps_a = ps_agg.tile([P, P], f32, tag="ps_a", name="ps_a")
                    nc.tensor.matmul(
                        out=ps_a[:, s * W0 : (s + 1) * W0],
                        lhsT=stage3[:, kk, :],
                        rhs=m0t3[:, kk, :],
                        start=(i == 0),
                        stop=(i == nb - 1),
                    )
                    if s == S0 - 1 and i == nb - 1:
                        # -------- supertile t epilogue --------
                        aggT = sp.tile([P, P], bf16, tag="aggT")
                        nc.vector.tensor_copy(out=aggT[:], in_=ps_a[:])
                        ps_o = ps_out.tile([P, H], f32, tag="ps_o", name="ps_o")
                        nc.tensor.matmul(
                            out=ps_o[:],
                            lhsT=xselfT_sb[:, t * P : (t + 1) * P],
                            rhs=ws0_sb[:],
                            start=True,
                            stop=False,
                        )
                        nc.tensor.matmul(
                            out=ps_o[:], lhsT=aggT[:], rhs=wn0_sb[:],
                            start=False, stop=not has_b0,
                        )
                        if has_b0:
                            nc.tensor.matmul(
                                out=ps_o[:], lhsT=ones_sb[:], rhs=brow["b0"][:],
                                start=False, stop=True,
                            )
                        h1p = sp.tile([P, H], bf16, tag="h1p")
                        nc.vector.tensor_scalar_max(h1p[:], ps_o[:], 0.0)
                        sq = sp.tile([P, H], bf16, tag="sq")
                        ss = sp.tile([P, 1], f32, tag="ss")
                        nc.vector.scalar_tensor_tensor(
                            out=sq[:], in0=h1p[:], scalar=0.0, in1=h1p[:],
                            op0=OP.bypass, op1=OP.mult, accum_out=ss[:],
                        )
                        nrm = sp.tile([P, 1], f32, tag="nrm")
                        nc.scalar.activation(nrm[:], ss[:], AT.Sqrt)
                        nrm2 = sp.tile([P, 1], f32, tag="nrm2")
                        nc.vector.tensor_scalar_max(nrm2[:], nrm[:], EPS_NORM)
                        rinv = sp.tile([P, 1], f32, tag="rinv")
                        nc.vector.reciprocal(rinv[:], nrm2[:])
                        h1n = sp.tile([P, H], bf16, tag="h1n")
                        nc.vector.tensor_scalar(
                            out=h1n[:], in0=h1p[:], scalar1=rinv[:, 0:1],
                            scalar2=None, op0=OP.mult,
                        )
                        nc.sync.dma_start(out=h1_d[t * P : (t + 1) * P, :], in_=h1n[:])
                        if t < LO_T0:
                            nc.sync.dma_start(
                                out=h1lo_d[t * P : (t + 1) * P, :], in_=h1n[:]
                            )
                        if t < T1:
                            for half in range(2):
                                hs = slice(half * P, (half + 1) * P)
                                ps_t = ps_tr.tile([P, P], bf16, tag="ps_t", name="ps_t")
                                nc.tensor.transpose(
                                    out=ps_t[:], in_=h1n[:, hs], identity=ident_sb[:]
                                )
                                nc.vector.tensor_copy(
                                    out=h1T_sb[:, half, t * P : (t + 1) * P], in_=ps_t[:]
                                )

            # ================= Layer 1 =================
            h1_ap = h1_d[:]
            h1lo_ap = h1lo_d[:]
            col_of_span = []
            col = 0
            for k0, n, _ in plan.spans:
                col_of_span.append(col)
                col += n * P // 16
            idx_sb = cp.tile([128, plan.idx_cols], i16)
            nc.sync.dma_start(out=idx_sb[:], in_=idx1_d[:])

            # early-partials stash: [f, tile * (2*128)] accumulated aggT halves
            earlyT = cp.tile([P, T1P * 2 * P], bf16)

            eT_list = [T for T in range(T1P) if plan.echunks[T]]
            eT_pos = 0
            lT_pos = 0
            bw = None
            bw_o = -1
            bw_cnt = 0
            # lT_pos thresholds at which each sub-RS fires
            rs_after = []
            acc = 0
            for _, n in RS_GROUPS:
                acc += NC * n
                rs_after.append(acc)
            rs_emitted = [False] * len(RS_GROUPS)
            span_id = 0
            stage_ref = {}  # global chunk id -> (stage3, m1t3, local col)

            def _emit_rs(k):
                nc.gpsimd.collective_compute(
                    kind="ReduceScatter",
                    op=mybir.AluOpType.add,
                    replica_groups=[list(range(NC))],
                    ins=[partials_g_d[k][:]],
                    outs=[rs_g_d[k][:]],
                )

            def _chain(chunks, ps1):
                for half in range(2):
                    for i, ck in enumerate(chunks):
                        s3, m3, kkl = stage_ref[ck]
                        nc.tensor.matmul(
                            out=ps1[:, half * P : (half + 1) * P],
                            lhsT=s3[:, kkl, half * P : (half + 1) * P],
                            rhs=m3[:, kkl, :],
                            start=(i == 0),
                            stop=(i == len(chunks) - 1),
                        )

            for g0 in range(0, C1, G1):
                gsz = min(G1, C1 - g0)
                stage = stage1p.tile([P, gsz * H], bf16, tag="stage1")
                stage3 = stage[:].rearrange("p (k f) -> p k f", f=H)
                m1t = metap.tile([P, gsz * P], bf16, tag="m1")
                m1t3 = m1t[:].rearrange("p (k w) -> p k w", w=P)
                nc.sync.dma_start(out=m1t3, in_=m1_d[:, g0 : g0 + gsz, :])

                done = 0
                while done < gsz:
                    k0, n, from_lo = plan.spans[span_id]
                    assert k0 == g0 + done, (k0, g0, done)
                    c0 = col_of_span[span_id]
                    nreg = nc.gpsimd.to_reg(n * P)
                    nc.gpsimd.dma_gather(
                        out_ap=stage3[:, done : done + n, :],
                        in_ap=h1lo_ap if from_lo else h1_ap,
                        idxs_ap=idx_sb[:, c0 : c0 + n * P // 16],
                        num_idxs=n * P,
                        num_idxs_reg=nreg,
                        elem_size=H,
                        queue_num=span_id % 2,
                    )
                    nc.gpsimd.free_register(nreg)
                    span_id += 1
                    done += n

                for kk in range(gsz):
                    stage_ref[g0 + kk] = (stage3, m1t3, kk)

                # E-phase: stash early partial sums per tile
                while (
                    eT_pos < len(eT_list)
                    and plan.echunks[eT_list[eT_pos]][-1] < g0 + gsz
                ):
                    T = eT_list[eT_pos]
                    ps1 = ps_out.tile([P, 2 * P], f32, tag="ps_o", name="ps1")
                    _chain(plan.echunks[T], ps1)
                    nc.vector.tensor_copy(
                        out=earlyT[:, T * 2 * P : (T + 1) * 2 * P], in_=ps1[:]
                    )
                    eT_pos += 1

                # L-phase: finish tiles in lateT order, batch-write partials
                if eT_pos == len(eT_list):
                    while lT_pos < T1P:
                        T = plan.lateT[lT_pos]
                        lcs = plan.lchunks[T]
                        if lcs and lcs[-1] >= g0 + gsz:
                            break
                        o, tt = T // T1, T % T1
                        k = plan.rs_group(tt)
                        gn = RS_GROUPS[k][1]
                        if bw is None:
                            bw = sp.tile([P, 3 * 2 * P], bf16, tag="bw")
                            bw_o = o
                            bw_cnt = 0
                        assert bw_o == o
                        slot = bw[:, bw_cnt * 2 * P : (bw_cnt + 1) * 2 * P]
                        est = earlyT[:, T * 2 * P : (T + 1) * 2 * P]
                        if lcs:
                            ps1 = ps_out.tile(
                                [P, 2 * P], f32, tag="ps_o", name="ps1"
                            )
                            _chain(lcs, ps1)
                            if plan.echunks[T]:
                                nc.vector.scalar_tensor_tensor(
                                    out=slot, in0=ps1[:], scalar=0.0, in1=est,
                                    op0=OP.bypass, op1=OP.add,
                                )
                            else:
                                nc.vector.tensor_copy(out=slot, in_=ps1[:])
                        else:
                            nc.vector.tensor_copy(out=slot, in_=est)
                        bw_cnt += 1
                        if bw_cnt == gn:
                            nc.sync.dma_start(
                                out=partials_g_d[k][bw_o],
                                in_=bw[:, : gn * 2 * P].rearrange(
                                    "p (t h d) -> p t h d", h=2, d=P
                                ),
                            )
                            bw = None
                        lT_pos += 1
                        for kk2, thr in enumerate(rs_after):
                            if lT_pos == thr and not rs_emitted[kk2]:
                                _emit_rs(kk2)
                                rs_emitted[kk2] = True

            assert eT_pos == len(eT_list) and lT_pos == T1P and bw is None
            assert all(rs_emitted)

            # ================= Layer 1 final + heads =================
            for tt in range(T1):
                rows = slice(tt * P, (tt + 1) * P)
                rw = sp.tile([P, 2 * P], bf16, tag="rw")
                k = plan.rs_group(tt)
                rs_src = rs_g_d[k][:, tt - RS_GROUPS[k][0]]
                nc.sync.dma_start(
                    out=rw[:].rearrange("p (h d) -> p h d", d=P), in_=rs_src
                )

                ps_f = ps_out.tile([P, H], f32, tag="ps_o", name="ps_f")
                nc.tensor.matmul(
                    out=ps_f[:], lhsT=h1T_sb[:, 0, rows], rhs=ws1_sb[0][:],
                    start=True, stop=False,
                )
                nc.tensor.matmul(
                    out=ps_f[:], lhsT=h1T_sb[:, 1, rows], rhs=ws1_sb[1][:],
                    start=False, stop=False,
                )
                nc.tensor.matmul(
                    out=ps_f[:], lhsT=rw[:, 0:P], rhs=wn1_sb[0][:],
                    start=False, stop=False,
                )
                nc.tensor.matmul(
                    out=ps_f[:], lhsT=rw[:, P : 2 * P], rhs=wn1_sb[1][:],
                    start=False, stop=not has_b1,
                )
                if has_b1:
                    nc.tensor.matmul(
                        out=ps_f[:], lhsT=ones_sb[:], rhs=brow["b1"][:],
                        start=False, stop=True,
                    )
                h2p = sp.tile([P, H], bf16, tag="h1p", name="h2p")
                nc.vector.tensor_scalar_max(h2p[:], ps_f[:], 0.0)
                sq = sp.tile([P, H], bf16, tag="sq", name="sq2")
                ss = sp.tile([P, 1], f32, tag="ss", name="ss2")
                nc.vector.scalar_tensor_tensor(
                    out=sq[:], in0=h2p[:], scalar=0.0, in1=h2p[:],
                    op0=OP.bypass, op1=OP.mult, accum_out=ss[:],
                )
                nrm = sp.tile([P, 1], f32, tag="nrm", name="nrm_2")
                nc.scalar.activation(nrm[:], ss[:], AT.Sqrt)
                nrm2 = sp.tile([P, 1], f32, tag="nrm2", name="nrm2_2")
                nc.vector.tensor_scalar_max(nrm2[:], nrm[:], EPS_NORM)
                rinv = sp.tile([P, 1], f32, tag="rinv", name="rinv2")
                nc.vector.reciprocal(rinv[:], nrm2[:])
                h2n = sp.tile([P, H], bf16, tag="h1n", name="h2n")
                nc.vector.tensor_scalar(
                    out=h2n[:], in0=h2p[:], scalar1=rinv[:, 0:1],
                    scalar2=None, op0=OP.mult,
                )

                h2T = []
                for half in range(2):
                    hs = slice(half * P, (half + 1) * P)
                    ps_t = ps_tr.tile([P, P], bf16, tag="ps_t", name="ps_t2")
                    nc.tensor.transpose(out=ps_t[:], in_=h2n[:, hs], identity=ident_sb[:])
                    hh = sp.tile([P, P], bf16, tag=f"h2T_{half}")
                    nc.vector.tensor_copy(out=hh[:], in_=ps_t[:])
                    h2T.append(hh)

                ps_zl = ps_agg.tile([P, L], f32, tag="ps_a", name="ps_zl")
                nc.tensor.matmul(
                    out=ps_zl[:], lhsT=h2T[0][:], rhs=wmu_sb[0][:], start=True, stop=False
                )
                nc.tensor.matmul(
                    out=ps_zl[:], lhsT=h2T[1][:], rhs=wmu_sb[1][:],
                    start=False, stop=not has_bmu,
                )
                if has_bmu:
                    nc.tensor.matmul(
                        out=ps_zl[:], lhsT=ones_sb[:], rhs=brow["b_mu"][:],
                        start=False, stop=True,
                    )
                zl_sb = sp.tile([P, L], f32, tag="zl")
                nc.vector.tensor_copy(out=zl_sb[:], in_=ps_zl[:])
                nc.sync.dma_start(out=zloc_d[rows, :], in_=zl_sb[:])

                ps_zs = ps_agg.tile([P, L], f32, tag="ps_a", name="ps_zs")
                nc.tensor.matmul(
                    out=ps_zs[:], lhsT=h2T[0][:], rhs=wvar_sb[0][:], start=True, stop=False
                )
                nc.tensor.matmul(
                    out=ps_zs[:], lhsT=h2T[1][:], rhs=wvar_sb[1][:],
                    start=False, stop=not has_bvar,
                )
                if has_bvar:
                    nc.tensor.matmul(
                        out=ps_zs[:], lhsT=ones_sb[:], rhs=brow["b_var"][:],
                        start=False, stop=True,
                    )
                zs_sb = sp.tile([P, L], f32, tag="zs")
                nc.scalar.activation(zs_sb[:], ps_zs[:], AT.Exp)
                nc.vector.tensor_scalar_add(zs_sb[:], zs_sb[:], 1e-6)
                nc.sync.dma_start(out=zscale_d[rows, :], in_=zs_sb[:])

    nc.compile()
    return nc


# ----------------------------------------------------------------------------
# Entry point
# ----------------------------------------------------------------------------
_CACHE = {}


def prepare(inputs):
    """Host preprocessing + program build.  Returns (nc, in_maps, postprocess)."""
    x = np.asarray(inputs["x"], np.float32)
    plan = _Plan(x, inputs["src0"], inputs["dst0"], inputs["src1"], inputs["dst1"])

    b0 = np.asarray(inputs["b0"], np.float32)
    b1 = np.asarray(inputs["b1"], np.float32)
    bmu = np.asarray(inputs["b_mu"], np.float32)
    bvar = np.asarray(inputs["b_var"], np.float32)
    has_b0, has_b1 = bool(np.any(b0)), bool(np.any(b1))
    has_bmu, has_bvar = bool(np.any(bmu)), bool(np.any(bvar))

    key = (plan.signature, has_b0, has_b1, has_bmu, has_bvar)
    if key not in _CACHE:
        _CACHE[key] = _build_program(plan, has_b0, has_b1, has_bmu, has_bvar)
    nc = _CACHE[key]

    def split2(w):
        w = np.asarray(w, np.float32)
        return np.stack([w[:P], w[P:]]).astype(npbf)

    common = {
        "ws0": np.asarray(inputs["W_self0"], np.float32).astype(npbf),
        "wn0": np.asarray(inputs["W_neigh0"], np.float32).astype(npbf),
        "ws1": split2(inputs["W_self1"]),
        "wn1": split2(inputs["W_neigh1"]),
        "wmu": split2(inputs["W_mu"]),
        "wvar": split2(inputs["W_var"]),
    }
    if has_b0:
        common["b0"] = b0
    if has_b1:
        common["b1"] = b1
    if has_bmu:
        common["b_mu"] = bmu
    if has_bvar:
        common["b_var"] = bvar

    in_maps = []
    for c in range(NC):
        m = dict(common)
        m["msgs0"] = plan.msgs0[c]
        m["m0"] = plan.m0[c]
        m["xselfT"] = plan.xselfT[c]
        m["m1"] = plan.m1[c]
        m["idx1"] = plan.idx1[c]
        in_maps.append(m)

    def postprocess(results):
        z_loc = np.empty((N2, L), np.float32)
        z_scale = np.empty((N2, L), np.float32)
        for c in range(NC):
            nodes = np.arange(c, N2, NC)
            pos = plan.pos_of[nodes]
            z_loc[nodes] = results[c]["z_loc"][pos]
            z_scale[nodes] = results[c]["z_scale"][pos]
        return z_loc, z_scale

    return nc, in_maps, postprocess


def kernel(**inputs):
    assert int(inputs.get("n_dst0", N1)) == N1 and int(inputs.get("n_dst1", N2)) == N2
    nc, in_maps, postprocess = prepare(inputs)
    res = run_bass_kernel_spmd(nc, in_maps, core_ids=list(range(NC)))
    return postprocess(res.results)


# revision 28
# speedup vs baseline: 1.0354x; 1.0354x over previous
"""Trainium2 Bass kernel for a 2-layer GraphSAGE(mean) encoder (8 NeuronCores).

v2 design (replaces the dma_gather-centric v1, which was bottlenecked by
SWDGE descriptor generation at ~8ns/row on the Q7s and by the GpSimd/DVE
shared-SBUF-port contention it induced):

  - Layer 0 (dst-partitioned by dst0 % 8): the host materializes each core's
    incoming-edge message rows (the "halo of remote src features" in edge-slot
    order) as a bf16 array plus the per-chunk one-hot segment-sum matrices M
    (mean weights folded in).  The device streams both sequentially (HWDGE),
    applies log1p on the Scalar engine, and accumulates aggT = msgs^T @ M on
    the PE per 32-dst sub-tile.  No gathers, no per-chunk DVE one-hot builds.
  - Layer 1 (edges by src1 % 8, dst1 in permuted layout): h1 is written to
    DRAM in bf16; a SWDGE dma_gather stages per-edge h1 rows; partial segment
    sums are computed in transposed [f, d] layout and ReduceScatter(add)
    delivers each core its own 1250 dst1 rows.  Final projection / relu /
    normalize / heads run per owning core; the host interleaves outputs.
  - All matmul operands are bf16 (fast weight load, 1 col/cycle); PSUM
    accumulation is f32.
"""

import math

import numpy as np

import concourse.bass as bass
import concourse.bacc as bacc
import concourse.mybir as mybir
from concourse.bass_utils import run_bass_kernel_spmd
from concourse.masks import make_identity
from concourse.tile import TileContext

# ----------------------------------------------------------------------------
# Problem constants (hardcoded; the harness always uses these shapes).
# ----------------------------------------------------------------------------
N0, N1, N2 = 200000, 50000, 10000
E0, E1 = 800000, 160000
F_IN, H, L = 128, 256, 32
NC = 8
P = 128

B1 = math.ceil(N2 // NC / P) * P  # 1280 padded per-core dst1 rows
T1 = B1 // P  # 10 final tiles per core
T1P = NC * T1  # 80 permuted partial tiles
RS_GROUPS = [(0, 5), (5, 5)]  # (tt0, n): pipelined sub-RS groups

# local h1 rows: positions [0, B1) hold the core's dst1 nodes (load-balanced
# permutation), the remaining dst0 nodes follow
T0 = math.ceil((B1 + N1 // NC - N2 // NC) / P) + 1  # 51 layer-0 supertiles
R0 = T0 * P  # 6528 padded local dst rows per core (slack eases balancing)
W0 = 32  # layer-0 M sub-tile width (dst cols per chunk)
S0 = P // W0  # 4 sub-tiles per supertile
NB0 = T0 * S0  # 200 layer-0 buckets

LO_T0 = 31  # h1 rows [0, LO_T0*128) are duplicated into h1_lo for early gathers
LO_ROWS = LO_T0 * P

G0 = 64  # layer-0 chunks per staging group
G1 = 16  # layer-1 chunks per staging group
GCH = 8  # chunks per dma_gather instruction (1024 idxs)

EPS_NORM = 1e-12

f32 = mybir.dt.float32
bf16 = mybir.dt.bfloat16
i16 = mybir.dt.int16
npbf = mybir.dt.np(bf16)


def _ranks_from_sorted(keys_sorted):
    """rank of each element within its equal-key run (keys_sorted ascending)."""
    n = keys_sorted.shape[0]
    if n == 0:
        return np.zeros(0, np.int64)
    new_run = np.empty(n, bool)
    new_run[0] = True
    new_run[1:] = keys_sorted[1:] != keys_sorted[:-1]
    starts = np.flatnonzero(new_run)
    run_ids = np.cumsum(new_run) - 1
    return np.arange(n) - starts[run_ids]


def _chunk_layout(counts, n_buckets):
    """counts: [NC, n_buckets] edge counts.  Returns (cap, base, total):
    cap[b] = chunks for bucket b (max over cores, >= 1), base = cumsum."""
    cap = np.maximum(np.ceil(counts / P).astype(np.int64).max(axis=0), 1)
    base = np.zeros(n_buckets + 1, np.int64)
    np.cumsum(cap, out=base[1:])
    return cap, base, int(base[-1])


def _wrap_idx(seg):
    """int16 flat idx list -> [128, len/16] wrapped layout for dma_gather."""
    ncols = len(seg) // 16
    return np.tile(seg.reshape(ncols, 16).T, (8, 1))


class _Plan:
    """Shared (cross-core) program structure + per-core input arrays."""

    def __init__(self, x, src0, dst0, src1, dst1):
        src0 = np.asarray(src0).astype(np.int64)
        dst0 = np.asarray(dst0).astype(np.int64)
        src1 = np.asarray(src1).astype(np.int64)
        dst1 = np.asarray(dst1).astype(np.int64)
        x = np.asarray(x, np.float32)

        deg0 = np.bincount(dst0, minlength=N1)
        inv0 = (1.0 / np.maximum(deg0, 1)).astype(np.float32)
        deg1 = np.bincount(dst1, minlength=N2)
        inv1 = (1.0 / np.maximum(deg1, 1)).astype(np.float32)

        # ------- load-balanced dst1 -> (tile, slot) position per owner ------
        core1 = src1 % NC
        nodevec = np.zeros((N2, NC), np.int64)
        np.add.at(nodevec, (dst1, core1), 1)
        pos_of = np.empty(N2, np.int64)
        for o in range(NC):
            nodes = np.arange(o, N2, NC)
            vec = nodevec[nodes]
            order_n = np.argsort(-vec.sum(1), kind="stable")
            fill = np.zeros(T1, np.int64)
            load = np.zeros((T1, NC), np.int64)
            tt_of = np.empty(len(nodes), np.int64)
            slot_of = np.empty(len(nodes), np.int64)
            for i in order_n:
                v = vec[i]
                score = (load + v).max(axis=1).astype(np.float64)
                score[fill >= P] = 1e18
                t = int(np.argmin(score))
                tt_of[i] = t
                slot_of[i] = fill[t]
                load[t] += v
                fill[t] += 1
            pos_of[nodes] = tt_of * P + slot_of
        self.pos_of = pos_of

        # reorder slots within each L1 tile so the tile's four layer-0
        # buckets carry balanced in-degree (slot order is free for L1)
        deg0n = deg0  # in-degree per dst0 node id
        for o in range(NC):
            nodes = np.arange(o, N2, NC)
            tts = pos_of[nodes] // P
            for tt in range(T1):
                sel = nodes[tts == tt]
                order_n = sel[np.argsort(-deg0n[sel], kind="stable")]
                loads = np.zeros(S0)
                fill = np.zeros(S0, np.int64)
                for g in order_n:
                    cand = loads + deg0n[g]
                    cand[fill >= W0] = np.inf
                    b = int(np.argmin(cand))
                    pos_of[g] = tt * P + b * W0 + fill[b]
                    loads[b] += deg0n[g]
                    fill[b] += 1

        # layer-0 local row of each dst0 node: dst1 nodes sit at their
        # position; the remaining dst0 nodes are LPT-balanced over the
        # 32-row buckets from row B1 so bucket in-degree stays under 4*128
        ldmap = np.empty(N1, np.int64)
        nbuck = (R0 - B1) // W0
        for c in range(NC):
            ds = np.arange(c, N1, NC)
            is1 = ds < N2
            ldmap[ds[is1]] = pos_of[ds[is1]]
            rest = ds[~is1]
            order_n = rest[np.argsort(-deg0n[rest], kind="stable")]
            loads = np.zeros(nbuck)
            fill = np.zeros(nbuck, np.int64)
            for g in order_n:
                cand = loads + deg0n[g]
                cand[fill >= W0] = np.inf
                b = int(np.argmin(cand))
                ldmap[g] = B1 + b * W0 + fill[b]
                loads[b] += deg0n[g]
                fill[b] += 1
        self.ldmap = ldmap

        # ---------------- layer 0 ----------------
        core0 = dst0 % NC
        ld0 = ldmap[dst0]
        b0 = ld0 // W0  # bucket in [0, NB0)
        counts0 = np.zeros((NC, NB0), np.int64)
        np.add.at(counts0, (core0, b0), 1)
        self.cap0, self.base0, self.C0 = _chunk_layout(counts0, NB0)

        order = np.lexsort((b0, core0))
        key = core0[order] * NB0 + b0[order]
        ranks = _ranks_from_sorted(key)
        kk = self.base0[b0[order]] + ranks // P
        pp = ranks % P

        self.msgs0 = np.zeros((NC, P, self.C0, F_IN), npbf)
        self.m0 = np.zeros((NC, P, self.C0, W0), npbf)
        co = core0[order]
        so = src0[order]
        do = dst0[order]
        ldo = ld0[order]
        gathered = x[so].astype(npbf)
        self.msgs0[co, pp, kk, :] = gathered
        self.m0[co, pp, kk, (ldo % W0)] = inv0[do]

        # per-core self rows, transposed: xselfT[c][f, ldmap[d]] = x[d, f]
        self.xselfT = np.zeros((NC, F_IN, R0), npbf)
        for c in range(NC):
            ds = np.arange(c, N1, NC)
            self.xselfT[c][:, ldmap[ds]] = x[ds].T.astype(npbf)

        # ---------------- layer 1 ----------------
        r1 = ldmap[src1]  # local h1 row on owning core
        o1 = dst1 % NC
        t1 = o1 * T1 + pos_of[dst1] // P  # permuted tile in [0, T1P)
        dloc1 = pos_of[dst1] % P
        counts1 = np.zeros((NC, T1P), np.int64)
        np.add.at(counts1, (core1, t1), 1)
        cap1, _, _ = _chunk_layout(counts1, T1P)
        cmax = int(cap1.max())

        # within each (core, tile), edges sorted by src row -> per-tile chunk
        # j holds the j-th lowest src rows; chunk max-rows ascend with j
        order = np.lexsort((r1, t1, core1))
        key = core1[order] * T1P + t1[order]
        ranks = _ranks_from_sorted(key)
        jj = ranks // P
        pp = ranks % P
        co = core1[order]
        to = t1[order]
        ro = r1[order]

        maxi = np.zeros((NC, T1P, cmax), np.int64)
        np.maximum.at(maxi, (co, to, jj), ro)
        maxi_sh = maxi.max(axis=0)  # [T1P, cmax] shared across cores
        e_cnt = np.zeros(T1P, np.int64)
        for T in range(T1P):
            n = int(cap1[T])
            # early = longest prefix of chunks whose rows all fit in h1_lo
            e_cnt[T] = int(
                (np.maximum.accumulate(maxi_sh[T, :n]) < LO_ROWS).sum()
            )

        # global chunk order: all early chunks (tile-major), then late chunks
        # ordered by RS group so each sub-ReduceScatter can fire early
        def rs_group(tt):
            for k, (tt0, n) in enumerate(RS_GROUPS):
                if tt0 <= tt < tt0 + n:
                    return k
            raise AssertionError(tt)

        self.rs_group = rs_group
        lateT = sorted(range(T1P), key=lambda T: (rs_group(T % T1), T))
        orderE = [(T, j) for T in range(T1P) for j in range(e_cnt[T])]
        orderL = [(T, j) for T in lateT for j in range(e_cnt[T], int(cap1[T]))]
        chunk_id = np.full((T1P, cmax), -1, np.int64)
        for g, (T, j) in enumerate(orderE + orderL):
            chunk_id[T, j] = g
        self.CE = len(orderE)
        self.C1 = len(orderE) + len(orderL)
        self.cap1 = cap1
        self.e_cnt = e_cnt
        self.lateT = lateT
        self.echunks = [
            [int(chunk_id[T, j]) for j in range(e_cnt[T])] for T in range(T1P)
        ]
        self.lchunks = [
            [int(chunk_id[T, j]) for j in range(e_cnt[T], int(cap1[T]))]
            for T in range(T1P)
        ]

        kk = chunk_id[to, jj]
        self.m1 = np.zeros((NC, P, self.C1, P), npbf)
        self.m1[co, pp, kk, dloc1[order]] = inv1[dst1[order]]

        idx_flat = np.zeros((NC, self.C1 * P), np.int16)
        idx_flat[co, kk * P + pp] = ro.astype(np.int16)

        # gather instructions: spans of <= GCH chunks, phase-pure, never
        # crossing a G1 staging-group boundary
        self.spans = []  # (k0, n, from_lo)
        for lo, hi, from_lo in ((0, self.CE, True), (self.CE, self.C1, False)):
            k0 = lo
            while k0 < hi:
                gend = (k0 // G1 + 1) * G1
                n = min(GCH, hi - k0, gend - k0)
                self.spans.append((k0, n, from_lo))
                k0 += n
        self.idx_cols = self.C1 * P // 16
        self.idx1 = np.zeros((NC, 128, self.idx_cols), np.int16)
        for c in range(NC):
            col = 0
            for k0, n, _ in self.spans:
                seg = idx_flat[c, k0 * P : (k0 + n) * P]
                self.idx1[c, :, col : col + n * P // 16] = _wrap_idx(seg)
                col += n * P // 16

        # ---------------- weights ----------------
        self.signature = (
            tuple(self.cap0.tolist()),
            tuple(self.cap1.tolist()),
            tuple(self.e_cnt.tolist()),
        )


# ----------------------------------------------------------------------------
# Program construction
# ----------------------------------------------------------------------------
def _build_program(plan, has_b0, has_b1, has_bmu, has_bvar):
    nc = bacc.Bacc(num_devices=NC, name="gnn_sage_v2", num_swdge_queues=2)

    C0, C1 = plan.C0, plan.C1
    msgs0_d = nc.dram_tensor("msgs0", (P, C0, F_IN), bf16, kind="ExternalInput")
    m0_d = nc.dram_tensor("m0", (P, C0, W0), bf16, kind="ExternalInput")
    xselfT_d = nc.dram_tensor("xselfT", (F_IN, R0), bf16, kind="ExternalInput")
    m1_d = nc.dram_tensor("m1", (P, C1, P), bf16, kind="ExternalInput")
    idx1_d = nc.dram_tensor("idx1", (128, plan.idx_cols), i16, kind="ExternalInput")
    ws0_d = nc.dram_tensor("ws0", (F_IN, H), bf16, kind="ExternalInput")
    wn0_d = nc.dram_tensor("wn0", (F_IN, H), bf16, kind="ExternalInput")
    ws1_d = nc.dram_tensor("ws1", (2, P, H), bf16, kind="ExternalInput")
    wn1_d = nc.dram_tensor("wn1", (2, P, H), bf16, kind="ExternalInput")
    wmu_d = nc.dram_tensor("wmu", (2, P, L), bf16, kind="ExternalInput")
    wvar_d = nc.dram_tensor("wvar", (2, P, L), bf16, kind="ExternalInput")
    b_d = {}
    if has_b0:
        b_d["b0"] = nc.dram_tensor("b0", (H,), f32, kind="ExternalInput")
    if has_b1:
        b_d["b1"] = nc.dram_tensor("b1", (H,), f32, kind="ExternalInput")
    if has_bmu:
        b_d["b_mu"] = nc.dram_tensor("b_mu", (L,), f32, kind="ExternalInput")
    if has_bvar:
        b_d["b_var"] = nc.dram_tensor("b_var", (L,), f32, kind="ExternalInput")

    h1_d = nc.dram_tensor("h1_scratch", (R0, H), bf16, kind="Internal")
    h1lo_d = nc.dram_tensor("h1_lo", (LO_ROWS, H), bf16, kind="Internal")
    partials_g_d = [
        nc.dram_tensor(f"s1_partials_{k}", (NC, P, n, 2, P), bf16, kind="Internal")
        for k, (_, n) in enumerate(RS_GROUPS)
    ]
    rs_g_d = [
        nc.dram_tensor(f"s1_reduced_{k}", (P, n, 2, P), bf16, kind="Internal")
        for k, (_, n) in enumerate(RS_GROUPS)
    ]

    zloc_d = nc.dram_tensor("z_loc", (B1, L), f32, kind="ExternalOutput")
    zscale_d = nc.dram_tensor("z_scale", (B1, L), f32, kind="ExternalOutput")

    AT = mybir.ActivationFunctionType
    OP = mybir.AluOpType

    # layer-0 chunk -> (supertile, subtile, index-in-bucket, bucket-size)
    chunk0_meta = []
    for b in range(NB0):
        nb = int(plan.cap0[b])
        for i in range(nb):
            chunk0_meta.append((b // S0, b % S0, i, nb))
    with TileContext(nc, num_cores=NC) as tc:
        with (
            tc.tile_pool(name="const", bufs=1) as cp,
            tc.tile_pool(name="stage0", bufs=2) as stagep,
            tc.tile_pool(name="mstage", bufs=2) as mp,
            tc.tile_pool(name="stage1", bufs=3) as stage1p,
            tc.tile_pool(name="meta", bufs=3) as metap,
            tc.tile_pool(name="small", bufs=4) as sp,
            tc.tile_pool(name="ps_agg", bufs=2, space="PSUM") as ps_agg,
            tc.tile_pool(name="ps_tr", bufs=2, space="PSUM") as ps_tr,
            tc.tile_pool(name="ps_out", bufs=2, space="PSUM") as ps_out,
        ):
            # ---- constants ----
            ident_sb = cp.tile([P, P], bf16)
            make_identity(nc, ident_sb[:])
            ws0_sb = cp.tile([P, H], bf16)
            nc.sync.dma_start(out=ws0_sb[:], in_=ws0_d[:])
            wn0_sb = cp.tile([P, H], bf16)
            nc.sync.dma_start(out=wn0_sb[:], in_=wn0_d[:])
            ws1_sb = [cp.tile([P, H], bf16, tag=f"ws1_{k}", name=f"ws1_{k}") for k in range(2)]
            wn1_sb = [cp.tile([P, H], bf16, tag=f"wn1_{k}", name=f"wn1_{k}") for k in range(2)]
            wmu_sb = [cp.tile([P, L], bf16, tag=f"wmu_{k}", name=f"wmu_{k}") for k in range(2)]
            wvar_sb = [cp.tile([P, L], bf16, tag=f"wvar_{k}", name=f"wvar_{k}") for k in range(2)]
            for k in range(2):
                nc.sync.dma_start(out=ws1_sb[k][:], in_=ws1_d[k])
                nc.sync.dma_start(out=wn1_sb[k][:], in_=wn1_d[k])
                nc.sync.dma_start(out=wmu_sb[k][:], in_=wmu_d[k])
                nc.sync.dma_start(out=wvar_sb[k][:], in_=wvar_d[k])
            if b_d:
                ones_sb = cp.tile([1, P], f32)
                nc.vector.memset(ones_sb[:], 1.0)
                brow = {}
                for name, hd in b_d.items():
                    t = cp.tile([1, hd.shape[0]], f32, tag=f"brow_{name}", name=f"brow_{name}")
                    nc.sync.dma_start(out=t[:], in_=hd[:].rearrange("n -> 1 n"))
                    brow[name] = t

            # xselfT: load + log1p once
            xselfT_sb = cp.tile([F_IN, R0], bf16)
            nc.sync.dma_start(out=xselfT_sb[:], in_=xselfT_d[:])
            nc.scalar.activation(xselfT_sb[:], xselfT_sb[:], AT.Ln, bias=1.0)

            # h1T stash for the final layer's self path
            h1T_sb = cp.tile([P, 2, B1], bf16)

            # ================= Layer 0 =================
            ps_a = None
            for g0 in range(0, C0, G0):
                gsz = min(G0, C0 - g0)
                stage = stagep.tile([P, gsz * F_IN], bf16, tag="stage0")
                stage3 = stage[:].rearrange("p (k f) -> p k f", f=F_IN)
                nc.sync.dma_start(out=stage3, in_=msgs0_d[:, g0 : g0 + gsz, :])
                m0t = mp.tile([P, gsz * W0], bf16, tag="m0")
                m0t3 = m0t[:].rearrange("p (k w) -> p k w", w=W0)
                nc.sync.dma_start(out=m0t3, in_=m0_d[:, g0 : g0 + gsz, :])
                nc.scalar.activation(stage[:], stage[:], AT.Ln, bias=1.0)

                for kk in range(gsz):
                    t, s, i, nb = chunk0_meta[g0 + kk]
                    if s == 0 and i == 0:
                        ps_a = ps_agg.tile([P, P], f32, tag="ps_a", name="ps_a")
                    nc.tensor.matmul(
                        out=ps_a[:, s * W0 : (s + 1) * W0],
                        lhsT=stage3[:, kk, :],
                        rhs=m0t3[:, kk, :],
                        start=(i == 0),
                        stop=(i == nb - 1),
                    )
                    if s == S0 - 1 and i == nb - 1:
                        # -------- supertile t epilogue --------
                        aggT = sp.tile([P, P], bf16, tag="aggT")
                        nc.vector.tensor_copy(out=aggT[:], in_=ps_a[:])
                        ps_o = ps_out.tile([P, H], f32, tag="ps_o", name="ps_o")
                        nc.tensor.matmul(
                            out=ps_o[:],
                            lhsT=xselfT_sb[:, t * P : (t + 1) * P],
                            rhs=ws0_sb[:],
                            start=True,
                            stop=False,
                        )
                        nc.tensor.matmul(
                            out=ps_o[:], lhsT=aggT[:], rhs=wn0_sb[:],
                            start=False, stop=not has_b0,
                        )
                        if has_b0:
                            nc.tensor.matmul(
                                out=ps_o[:], lhsT=ones_sb[:], rhs=brow["b0"][:],
                                start=False, stop=True,
                            )
                        h1p = sp.tile([P, H], bf16, tag="h1p")
                        nc.vector.tensor_scalar_max(h1p[:], ps_o[:], 0.0)
                        sq = sp.tile([P, H], bf16, tag="sq")
                        ss = sp.tile([P, 1], f32, tag="ss")
                        nc.scalar.activation(sq[:], h1p[:], AT.Square, accum_out=ss[:])
                        nrm = sp.tile([P, 1], f32, tag="nrm")
                        nc.scalar.activation(nrm[:], ss[:], AT.Sqrt)
                        nrm2 = sp.tile([P, 1], f32, tag="nrm2")
                        nc.vector.tensor_scalar_max(nrm2[:], nrm[:], EPS_NORM)
                        rinv = sp.tile([P, 1], f32, tag="rinv")
                        nc.vector.reciprocal(rinv[:], nrm2[:])
                        h1n = sp.tile([P, H], bf16, tag="h1n")
                        nc.vector.tensor_scalar(
                            out=h1n[:], in0=h1p[:], scalar1=rinv[:, 0:1],
                            scalar2=None, op0=OP.mult,
                        )
                        nc.sync.dma_start(out=h1_d[t * P : (t + 1) * P, :], in_=h1n[:])
                        if t < LO_T0:
                            nc.sync.dma_start(
                                out=h1lo_d[t * P : (t + 1) * P, :], in_=h1n[:]
                            )
                        if t < T1:
                            for half in range(2):
                                hs = slice(half * P, (half + 1) * P)
                                ps_t = ps_tr.tile([P, P], bf16, tag="ps_t", name="ps_t")
                                nc.tensor.transpose(
                                    out=ps_t[:], in_=h1n[:, hs], identity=ident_sb[:]
                                )
                                nc.vector.tensor_copy(
                                    out=h1T_sb[:, half, t * P : (t + 1) * P], in_=ps_t[:]
                                )

            # ================= Layer 1 =================
            h1_ap = h1_d[:]
            h1lo_ap = h1lo_d[:]
            col_of_span = []
            col = 0
            for k0, n, _ in plan.spans:
                col_of_span.append(col)
                col += n * P // 16
            idx_sb = cp.tile([128, plan.idx_cols], i16)
            nc.sync.dma_start(out=idx_sb[:], in_=idx1_d[:])

            # early-partials stash: [f, tile * (2*128)] accumulated aggT halves
            earlyT = cp.tile([P, T1P * 2 * P], bf16)

            eT_list = [T for T in range(T1P) if plan.echunks[T]]
            eT_pos = 0
            lT_pos = 0
            bw = None
            bw_o = -1
            bw_cnt = 0
            # lT_pos thresholds at which each sub-RS fires
            rs_after = []
            acc = 0
            for _, n in RS_GROUPS:
                acc += NC * n
                rs_after.append(acc)
            rs_emitted = [False] * len(RS_GROUPS)
            span_id = 0
            stage_ref = {}  # global chunk id -> (stage3, m1t3, local col)

            def _emit_rs(k):
                nc.gpsimd.collective_compute(
                    kind="ReduceScatter",
                    op=mybir.AluOpType.add,
                    replica_groups=[list(range(NC))],
                    ins=[partials_g_d[k][:]],
                    outs=[rs_g_d[k][:]],
                )

            def _chain(chunks, ps1):
                for half in range(2):
                    for i, ck in enumerate(chunks):
                        s3, m3, kkl = stage_ref[ck]
                        nc.tensor.matmul(
                            out=ps1[:, half * P : (half + 1) * P],
                            lhsT=s3[:, kkl, half * P : (half + 1) * P],
                            rhs=m3[:, kkl, :],
                            start=(i == 0),
                            stop=(i == len(chunks) - 1),
                        )

            for g0 in range(0, C1, G1):
                gsz = min(G1, C1 - g0)
                stage = stage1p.tile([P, gsz * H], bf16, tag="stage1")
                stage3 = stage[:].rearrange("p (k f) -> p k f", f=H)
                m1t = metap.tile([P, gsz * P], bf16, tag="m1")
                m1t3 = m1t[:].rearrange("p (k w) -> p k w", w=P)
                nc.sync.dma_start(out=m1t3, in_=m1_d[:, g0 : g0 + gsz, :])

                done = 0
                while done < gsz:
                    k0, n, from_lo = plan.spans[span_id]
                    assert k0 == g0 + done, (k0, g0, done)
                    c0 = col_of_span[span_id]
                    nreg = nc.gpsimd.to_reg(n * P)
                    nc.gpsimd.dma_gather(
                        out_ap=stage3[:, done : done + n, :],
                        in_ap=h1lo_ap if from_lo else h1_ap,
                        idxs_ap=idx_sb[:, c0 : c0 + n * P // 16],
                        num_idxs=n * P,
                        num_idxs_reg=nreg,
                        elem_size=H,
                        queue_num=span_id % 2,
                    )
                    nc.gpsimd.free_register(nreg)
                    span_id += 1
                    done += n

                for kk in range(gsz):
                    stage_ref[g0 + kk] = (stage3, m1t3, kk)

                # E-phase: stash early partial sums per tile
                while (
                    eT_pos < len(eT_list)
                    and plan.echunks[eT_list[eT_pos]][-1] < g0 + gsz
                ):
                    T = eT_list[eT_pos]
                    ps1 = ps_out.tile([P, 2 * P], f32, tag="ps_o", name="ps1")
                    _chain(plan.echunks[T], ps1)
                    nc.vector.tensor_copy(
                        out=earlyT[:, T * 2 * P : (T + 1) * 2 * P], in_=ps1[:]
                    )
                    eT_pos += 1

                # L-phase: finish tiles in lateT order, batch-write partials
                if eT_pos == len(eT_list):
                    while lT_pos < T1P:
                        T = plan.lateT[lT_pos]
                        lcs = plan.lchunks[T]
                        if lcs and lcs[-1] >= g0 + gsz:
                            break
                        o, tt = T // T1, T % T1
                        k = plan.rs_group(tt)
                        gn = RS_GROUPS[k][1]
                        if bw is None:
                            bw = sp.tile([P, 5 * 2 * P], bf16, tag="bw")
                            bw_o = o
                            bw_cnt = 0
                        assert bw_o == o
                        slot = bw[:, bw_cnt * 2 * P : (bw_cnt + 1) * 2 * P]
                        est = earlyT[:, T * 2 * P : (T + 1) * 2 * P]
                        if lcs:
                            ps1 = ps_out.tile(
                                [P, 2 * P], f32, tag="ps_o", name="ps1"
                            )
                            _chain(lcs, ps1)
                            if plan.echunks[T]:
                                nc.vector.scalar_tensor_tensor(
                                    out=slot, in0=ps1[:], scalar=0.0, in1=est,
                                    op0=OP.bypass, op1=OP.add,
                                )
                            else:
                                nc.vector.tensor_copy(out=slot, in_=ps1[:])
                        else:
                            nc.vector.tensor_copy(out=slot, in_=est)
                        bw_cnt += 1
                        if bw_cnt == gn:
                            nc.sync.dma_start(
                                out=partials_g_d[k][bw_o],
                                in_=bw[:, : gn * 2 * P].rearrange(
                                    "p (t h d) -> p t h d", h=2, d=P
                                ),
                            )
                            bw = None
                        lT_pos += 1
                        for kk2, thr in enumerate(rs_after):
                            if lT_pos == thr and not rs_emitted[kk2]:
                                _emit_rs(kk2)
                                rs_emitted[kk2] = True

            assert eT_pos == len(eT_list) and lT_pos == T1P and bw is None
            assert all(rs_emitted)

            # ================= Layer 1 final + heads =================
            for tt in range(T1):
                rows = slice(tt * P, (tt + 1) * P)
                rw = sp.tile([P, 2 * P], bf16, tag="rw")
                k = plan.rs_group(tt)
                rs_src = rs_g_d[k][:, tt - RS_GROUPS[k][0]]
                nc.sync.dma_start(
                    out=rw[:].rearrange("p (h d) -> p h d", d=P), in_=rs_src
                )

                ps_f = ps_out.tile([P, H], f32, tag="ps_o", name="ps_f")
                nc.tensor.matmul(
                    out=ps_f[:], lhsT=h1T_sb[:, 0, rows], rhs=ws1_sb[0][:],
                    start=True, stop=False,
                )
                nc.tensor.matmul(
                    out=ps_f[:], lhsT=h1T_sb[:, 1, rows], rhs=ws1_sb[1][:],
                    start=False, stop=False,
                )
                nc.tensor.matmul(
                    out=ps_f[:], lhsT=rw[:, 0:P], rhs=wn1_sb[0][:],
                    start=False, stop=False,
                )
                nc.tensor.matmul(
                    out=ps_f[:], lhsT=rw[:, P : 2 * P], rhs=wn1_sb[1][:],
                    start=False, stop=not has_b1,
                )
                if has_b1:
                    nc.tensor.matmul(
                        out=ps_f[:], lhsT=ones_sb[:], rhs=brow["b1"][:],
                        start=False, stop=True,
                    )
                h2p = sp.tile([P, H], bf16, tag="h1p", name="h2p")
                nc.vector.tensor_scalar_max(h2p[:], ps_f[:], 0.0)
                sq = sp.tile([P, H], bf16, tag="sq", name="sq2")
                ss = sp.tile([P, 1], f32, tag="ss", name="ss2")
                nc.scalar.activation(sq[:], h2p[:], AT.Square, accum_out=ss[:])
                nrm = sp.tile([P, 1], f32, tag="nrm", name="nrm_2")
                nc.scalar.activation(nrm[:], ss[:], AT.Sqrt)
                nrm2 = sp.tile([P, 1], f32, tag="nrm2", name="nrm2_2")
                nc.vector.tensor_scalar_max(nrm2[:], nrm[:], EPS_NORM)
                rinv = sp.tile([P, 1], f32, tag="rinv", name="rinv2")
                nc.vector.reciprocal(rinv[:], nrm2[:])
                h2n = sp.tile([P, H], bf16, tag="h1n", name="h2n")
                nc.vector.tensor_scalar(
                    out=h2n[:], in0=h2p[:], scalar1=rinv[:, 0:1],
                    scalar2=None, op0=OP.mult,
                )

                h2T = []
                for half in range(2):
                    hs = slice(half * P, (half + 1) * P)
                    ps_t = ps_tr.tile([P, P], bf16, tag="ps_t", name="ps_t2")
                    nc.tensor.transpose(out=ps_t[:], in_=h2n[:, hs], identity=ident_sb[:])
                    hh = sp.tile([P, P], bf16, tag=f"h2T_{half}")
                    nc.vector.tensor_copy(out=hh[:], in_=ps_t[:])
                    h2T.append(hh)

                ps_zl = ps_agg.tile([P, L], f32, tag="ps_a", name="ps_zl")
                nc.tensor.matmul(
                    out=ps_zl[:], lhsT=h2T[0][:], rhs=wmu_sb[0][:], start=True, stop=False
                )
                nc.tensor.matmul(
                    out=ps_zl[:], lhsT=h2T[1][:], rhs=wmu_sb[1][:],
                    start=False, stop=not has_bmu,
                )
                if has_bmu:
                    nc.tensor.matmul(
                        out=ps_zl[:], lhsT=ones_sb[:], rhs=brow["b_mu"][:],
                        start=False, stop=True,
                    )
                zl_sb = sp.tile([P, L], f32, tag="zl")
                nc.vector.tensor_copy(out=zl_sb[:], in_=ps_zl[:])
                nc.sync.dma_start(out=zloc_d[rows, :], in_=zl_sb[:])

                ps_zs = ps_agg.tile([P, L], f32, tag="ps_a", name="ps_zs")
                nc.tensor.matmul(
                    out=ps_zs[:], lhsT=h2T[0][:], rhs=wvar_sb[0][:], start=True, stop=False
                )
                nc.tensor.matmul(
                    out=ps_zs[:], lhsT=h2T[1][:], rhs=wvar_sb[1][:],
                    start=False, stop=not has_bvar,
                )
                if has_bvar:
                    nc.tensor.matmul(
                        out=ps_zs[:], lhsT=ones_sb[:], rhs=brow["b_var"][:],
                        start=False, stop=True,
                    )
                zs_sb = sp.tile([P, L], f32, tag="zs")
                nc.scalar.activation(zs_sb[:], ps_zs[:], AT.Exp)
                nc.vector.tensor_scalar_add(zs_sb[:], zs_sb[:], 1e-6)
                nc.sync.dma_start(out=zscale_d[rows, :], in_=zs_sb[:])

    nc.compile()
    return nc


# ----------------------------------------------------------------------------
# Entry point
# ----------------------------------------------------------------------------
_CACHE = {}


def prepare(inputs):
    """Host preprocessing + program build.  Returns (nc, in_maps, postprocess)."""
    x = np.asarray(inputs["x"], np.float32)
    plan = _Plan(x, inputs["src0"], inputs["dst0"], inputs["src1"], inputs["dst1"])

    b0 = np.asarray(inputs["b0"], np.float32)
    b1 = np.asarray(inputs["b1"], np.float32)
    bmu = np.asarray(inputs["b_mu"], np.float32)
    bvar = np.asarray(inputs["b_var"], np.float32)
    has_b0, has_b1 = bool(np.any(b0)), bool(np.any(b1))
    has_bmu, has_bvar = bool(np.any(bmu)), bool(np.any(bvar))

    key = (plan.signature, has_b0, has_b1, has_bmu, has_bvar)
    if key not in _CACHE:
        _CACHE[key] = _build_program(plan, has_b0, has_b1, has_bmu, has_bvar)
    nc = _CACHE[key]

    def split2(w):
        w = np.asarray(w, np.float32)
        return np.stack([w[:P], w[P:]]).astype(npbf)

    common = {
        "ws0": np.asarray(inputs["W_self0"], np.float32).astype(npbf),
        "wn0": np.asarray(inputs["W_neigh0"], np.float32).astype(npbf),
        "ws1": split2(inputs["W_self1"]),
        "wn1": split2(inputs["W_neigh1"]),
        "wmu": split2(inputs["W_mu"]),
        "wvar": split2(inputs["W_var"]),
    }
    if has_b0:
        common["b0"] = b0
    if has_b1:
        common["b1"] = b1
    if has_bmu:
        common["b_mu"] = bmu
    if has_bvar:
        common["b_var"] = bvar

    in_maps = []
    for c in range(NC):
        m = dict(common)
        m["msgs0"] = plan.msgs0[c]
        m["m0"] = plan.m0[c]
        m["xselfT"] = plan.xselfT[c]
        m["m1"] = plan.m1[c]
        m["idx1"] = plan.idx1[c]
        in_maps.append(m)

    def postprocess(results):
        z_loc = np.empty((N2, L), np.float32)
        z_scale = np.empty((N2, L), np.float32)
        for c in range(NC):
            nodes = np.arange(c, N2, NC)
            pos = plan.pos_of[nodes]
            z_loc[nodes] = results[c]["z_loc"][pos]
            z_scale[nodes] = results[c]["z_scale"][pos]
        return z_loc, z_scale

    return nc, in_maps, postprocess


def kernel(**inputs):
    assert int(inputs.get("n_dst0", N1)) == N1 and int(inputs.get("n_dst1", N2)) == N2
    nc, in_maps, postprocess = prepare(inputs)
    res = run_bass_kernel_spmd(nc, in_maps, core_ids=list(range(NC)))
    return postprocess(res.results)
